# revision 1
# baseline (speedup 1.0000x reference)
"""Trainium2 Bass kernel for nn_InvariantCrossAttention.

Math: the reference computes softmax(-(Q2_i + K2_j), axis=j) — but -Q2_i is
constant along the softmax axis, so it cancels. The attention row is the same
for every query i, hence context[b,i] is i-independent and the final mean over
N is a no-op:

    out[b] = sum_j exp(-K2[b,j]) * K2[b,j] / sum_j exp(-K2[b,j])
    K2[b,j] = (x[b,j] - mean_j x[b,:])^2,  x = all_atom_features[:, :, 0]

cdr3_features does not affect the output (for any input values). The kernel
computes the reduction above on-device. Sharding: the post-simplification
problem is 128KB of input and ~20 instructions, so every core runs the full
(replicated) computation and core 0's output is returned — any cross-core
split would put a collective (multi-us) on a sub-us critical path.

Layout: x viewed as [128 partitions, 256 cols]; partition p holds batch p//32
(32 partitions per batch, contiguous 1KB rows -> full DMA bandwidth).
Cross-partition per-batch reduce/broadcast are tiny PE matmuls against
memset-generated group masks. The input load is split across both HWDGE
rings (SP + Activation) so the two halves' completion latencies overlap, and
the per-batch sum consumes each half directly via PSUM accumulation.
"""

import os

import numpy as np

B = 4  # batch
M = 8192  # all_atom length (softmax axis)
P = 128  # SBUF partitions
COLS = B * M // P  # 256 elements per partition
PPB = P // B  # 32 partitions per batch
N_CORES = 8

_cache = {}
last_results = None  # BassKernelResults of the most recent run (for test.py)


def _build():
    import concourse.bacc as bacc
    import concourse.bass as bass
    import concourse.mybir as mybir
    import concourse.tile as tile

    f32 = mybir.dt.float32
    bf16 = mybir.dt.bfloat16
    nc = bacc.Bacc("TRN2", target_bir_lowering=False, debug=False)

    x_dram = nc.dram_tensor("x", [P, COLS], f32, kind="ExternalInput")
    nmaskT_dram = nc.dram_tensor("nmaskT", [B, P], bf16, kind="ExternalInput")
    out_dram = nc.dram_tensor("out", [B, 1], f32, kind="ExternalOutput")

    with tile.TileContext(nc) as tc:
        with (
            tc.tile_pool(name="sbuf", bufs=1) as pool,
            tc.tile_pool(name="psum", bufs=1, space=bass.MemorySpace.PSUM) as psum,
        ):
            X = pool.tile([P, COLS], f32)
            mask = pool.tile([P, B], f32)
            nmaskT = pool.tile([B, P], bf16)
            zb = pool.tile([P, 1], f32)

            nc.gpsimd.memset(zb[:], 0.0)
            # mask[p,b] = 1 iff p//32 == b, built with quadrant-aligned
            # memsets so no constant DMA delays the input load's sems.
            nc.vector.memset(mask[:], 0.0)
            for b in range(B):
                nc.vector.memset(mask[b * PPB : (b + 1) * PPB, b : b + 1], 1.0)

            # Input halves get the two HWDGE rings (SP + Activation) first;
            # nmaskT (not memset-buildable: partition offsets 1..3 are not
            # quadrant-aligned) pipelines behind X_h1 on the Scalar ring.
            H = P // 2
            nc.sync.dma_start(X[0:H, :], x_dram[0:H, :])
            nc.scalar.dma_start(X[H:P, :], x_dram[H:P, :])
            nc.scalar.dma_start(nmaskT[:], nmaskT_dram[:])

            partial = pool.tile([P, 1], f32)
            nc.vector.reduce_sum(partial[:], X[:], axis=mybir.AxisListType.X)

            # Per-batch sums then negative-mean broadcast via tiny PE matmuls.
            S1 = psum.tile([B, 1], f32)
            nc.tensor.matmul(S1[:], mask[:], partial[:])
            s4 = pool.tile([B, 1], bf16)
            nc.vector.tensor_copy(s4[:], S1[:])
            NM = psum.tile([P, 1], f32)
            nc.tensor.matmul(NM[:], nmaskT[:], s4[:])
            nm = pool.tile([P, 1], f32)
            nc.vector.tensor_copy(nm[:], NM[:])

            # K2 = (x - mean)^2; w = exp(-K2) with per-partition sum;
            # wk = w*K2 with per-partition sum; mask.T @ [s1|s2] -> [4,2].
            K2 = pool.tile([P, COLS], f32)
            nc.scalar.activation(
                K2[:], X[:], mybir.ActivationFunctionType.Square, bias=nm[:]
            )

            partials = pool.tile([P, 2], f32)
            w = pool.tile([P, COLS], f32)
            nc.scalar.activation(
                w[:],
                K2[:],
                mybir.ActivationFunctionType.Exp,
                bias=zb[:],
                scale=-1.0,
                accum_out=partials[:, 0:1],
            )

            wk = pool.tile([P, COLS], f32)
            nc.vector.scalar_tensor_tensor(
                wk[:],
                w[:],
                1.0,
                K2[:],
                op0=mybir.AluOpType.mult,
                op1=mybir.AluOpType.mult,
                accum_out=partials[:, 1:2],
            )

            S2 = psum.tile([B, 2], f32)
            nc.tensor.matmul(
                S2[:], mask[:], partials[:]
            )

            r = pool.tile([B, 1], f32)
            nc.vector.reciprocal(r[:], S2[:, 0:1])
            res = pool.tile([B, 1], f32)
            nc.vector.tensor_tensor(
                res[:], S2[:, 1:2], r[:], op=mybir.AluOpType.mult
            )

            nc.sync.dma_start(out_dram[:], res[:])

    nc.compile()
    return nc


def kernel(cdr3_features=None, all_atom_features=None, **_unused):
    from concourse.bass_utils import run_bass_kernel_spmd

    global last_results
    if "nc" not in _cache:
        _cache["nc"] = _build()
    nc = _cache["nc"]

    x = np.ascontiguousarray(np.asarray(all_atom_features, dtype=np.float32)).reshape(
        P, COLS
    )
    import ml_dtypes

    nmaskT = np.zeros((B, P), ml_dtypes.bfloat16)
    for b in range(B):
        nmaskT[b, b * PPB : (b + 1) * PPB] = ml_dtypes.bfloat16(-1.0 / M)
    in_map = {"x": x, "nmaskT": nmaskT}

    trace = bool(os.environ.get("KERNEL_TRACE"))
    last_results = run_bass_kernel_spmd(
        nc, [in_map] * N_CORES, list(range(N_CORES)), trace=trace
    )
    out = np.asarray(last_results.results[0]["out"], dtype=np.float32)
    return out.reshape(B, 1)



# revision 12
# speedup vs baseline: 1.0900x; 1.0900x over previous
"""Trainium2 Bass kernel for nn_InvariantCrossAttention.

Math: the reference computes softmax(-(Q2_i + K2_j), axis=j) — but -Q2_i is
constant along the softmax axis, so it cancels. The attention row is the same
for every query i, hence context[b,i] is i-independent and the final mean over
N is a no-op:

    out[b] = sum_j exp(-K2[b,j]) * K2[b,j] / sum_j exp(-K2[b,j])
    K2[b,j] = (x[b,j] - mean_j x[b,:])^2,  x = all_atom_features[:, :, 0]

cdr3_features does not affect the output (for any input values). Every core
runs the full (replicated) computation — a cross-core split would put a
multi-us collective on a sub-us critical path.

This version is raw Bass (no TileContext): the profiler's measured window
starts at the first BIR-named instruction and ends at the end of the NEFF's
fixed semaphore-reset epilogue, so the Tile preamble (const memsets, barrier,
~1.2us) was pure overhead. Structure:

  - x viewed as [128 part, 256 cols]; partition p holds batch p//32.
  - Input DMA split across the two HWDGE rings (SP + Activation).
  - Per-batch -mean lands per-partition via ONE matmul against a memset-built
    block-diagonal [128,128] bf16 constant (value -1/8192 exactly).
  - exp(-t^2) comes from one Derivative_Erf activation (= 2/sqrt(pi)*e^{-t^2};
    the constant cancels in the ratio), with fused per-partition accumulation
    for sum(w). DVE computes t^2 and w*t^2 (accumulating sum(w*t^2)) in
    parallel with the Scalar engine.
  - Final per-batch sums via one matmul with the accumulator columns as the
    stationary operand, giving [2,4] in PSUM: row 0 = sum(w), row 1 = sum(wk),
    batches along the free dim, so the result lives on one partition and the
    output DMA is a single contiguous 16B packet.
  - No explicit wait on the output DMA: the NEFF epilogue's post-barrier queue
    DRAIN covers it after the (longer) semaphore-reset tail.
"""

import os

import numpy as np

B = 4  # batch
M = 8192  # all_atom length (softmax axis)
P = 128  # SBUF partitions
COLS = B * M // P  # 256 elements per partition
PPB = P // B  # 32 partitions per batch
N_CORES = 8

_cache = {}
last_results = None  # BassKernelResults of the most recent run (for test.py)


def _build():
    import concourse.bacc as bacc
    import concourse.bass as bass
    import concourse.mybir as mybir

    f32 = mybir.dt.float32
    bf16 = mybir.dt.bfloat16
    AF = mybir.ActivationFunctionType
    ALU = mybir.AluOpType
    nc = bacc.Bacc("TRN2", target_bir_lowering=False, debug=False)

    x_dram = nc.dram_tensor("x", [P, COLS], f32, kind="ExternalInput")
    out_dram = nc.dram_tensor("out", [1, B], f32, kind="ExternalOutput")

    H = P // 2
    from contextlib import ExitStack

    with ExitStack() as es:
        X = es.enter_context(nc.sbuf_tensor([P, COLS], f32))
        BO = es.enter_context(nc.sbuf_tensor([P, P], bf16))  # block-diag -1/M
        MK = es.enter_context(nc.sbuf_tensor([P, B], bf16))  # block mask (ones)
        ps = es.enter_context(nc.sbuf_tensor([P, 1], bf16))  # per-part col sums
        nm = es.enter_context(nc.sbuf_tensor([P, 1], f32))  # -mean per partition
        w = es.enter_context(nc.sbuf_tensor([P, COLS], bf16))  # ~exp(-t^2)
        t = es.enter_context(nc.sbuf_tensor([P, COLS], bf16))  # x - mean
        t2 = es.enter_context(nc.sbuf_tensor([P, COLS], bf16))  # t^2
        wk = es.enter_context(nc.sbuf_tensor([P, COLS], bf16))  # w * t^2
        acc = es.enter_context(nc.sbuf_tensor([P, 2], f32))  # [sum w | sum wk]
        accb = es.enter_context(nc.sbuf_tensor([P, 2], bf16))  # bf16 for matmul
        rcp = es.enter_context(nc.sbuf_tensor([1, B], f32))  # 1/sum(w)
        res = es.enter_context(nc.sbuf_tensor([1, B], f32))  # final out
        NM = es.enter_context(nc.psum_tensor([P, 1], f32))
        SW = es.enter_context(nc.psum_tensor([1, B], f32))  # sum(w) per batch
        SK = es.enter_context(nc.psum_tensor([1, B], f32))  # sum(wk) per batch
        dS = es.enter_context(nc.semaphore())  # input DMA completion (+16/half)
        vD = es.enter_context(nc.semaphore())  # DVE same-engine completion chain
        vR = es.enter_context(nc.semaphore())  # reduce done
        tN = es.enter_context(nc.semaphore())  # NM matmul done
        vC = es.enter_context(nc.semaphore())  # nm copy done
        sW = es.enter_context(nc.semaphore())  # DErf (w + acc col0) done
        vK = es.enter_context(nc.semaphore())  # both acc cols cast to bf16
        tS = es.enter_context(nc.semaphore())  # both sum matmuls done
        vF = es.enter_context(nc.semaphore())  # res ready
        dO = es.enter_context(nc.semaphore())  # output DMA completion (unwaited)
        with nc.Block(no_gpsimd_drain=True) as block:

            @block.sync
            def _(sync):
                sync.dma_start(X[0:H, :], x_dram[0:H, :]).then_inc(dS, 16)
                sync.wait_ge(vF, 1)
                # completion sem required by the descriptor; never waited on —
                # the NEFF epilogue's queue DRAIN covers output landing.
                sync.dma_start(out_dram[:, :], res[:, :]).then_inc(dO, 16)

            @block.scalar
            def _(scalar):
                scalar.dma_start(X[H:P, :], x_dram[H:P, :]).then_inc(dS, 16)
                scalar.wait_ge(vC, 1)
                # then_inc lands on the lowered ACTIVATION_READ_ACCUMULATOR,
                # so sW also covers acc[:, 0].
                scalar.activation(
                    w[:], X[:], AF.Derivative_Erf, bias=nm[:], accum_out=acc[:, 0:1]
                ).then_inc(sW, 1)

            @block.vector
            def _(vector):
                # Constants, built while the input DMA is in flight. Engines
                # complete out of order, so same-engine RAW/WAW deps are
                # chained through vD (waits fuse into the next instruction).
                # BO row-bands are mutually disjoint: no chaining among them.
                for b in range(B):
                    vector.memset(BO[b * PPB : (b + 1) * PPB, :], 0.0).then_inc(
                        vD, 1
                    )
                for b in range(B):
                    vector.wait_ge(vD, B)
                    vector.memset(
                        BO[b * PPB : (b + 1) * PPB, b * PPB : (b + 1) * PPB],
                        -1.0 / M,
                    ).then_inc(vD, 1)
                vector.memset(MK[:], 0.0).then_inc(vD, 1)
                for b in range(B):
                    vector.wait_ge(vD, 2 * B + 1)
                    vector.memset(
                        MK[b * PPB : (b + 1) * PPB, b : b + 1], 1.0
                    ).then_inc(vD, 1)

                vector.wait_ge(dS, 32)
                vector.wait_ge(vD, 3 * B + 1)  # all memsets retired
                with nc.allow_low_precision(reason="col sums feed a bf16 matmul"):
                    vector.tensor_reduce(
                        ps[:], X[:], axis=mybir.AxisListType.X, op=ALU.add
                    ).then_inc(vR, 1)
                vector.wait_ge(tN, 1)
                vector.tensor_copy(nm[:], NM[:]).then_inc(vC, 1)
                vector.wait_ge(vC, 1)  # own copy retired before reading nm
                vector.tensor_scalar_add(t[:], X[:], nm[:]).then_inc(vD, 1)
                vector.wait_ge(vD, 3 * B + 2)
                vector.tensor_tensor(t2[:], t[:], t[:], op=ALU.mult).then_inc(vD, 1)
                vector.wait_ge(sW, 1)
                vector.wait_ge(vD, 3 * B + 3)
                vector.scalar_tensor_tensor(
                    wk[:], w[:], 1.0, t2[:],
                    op0=ALU.mult, op1=ALU.mult, accum_out=acc[:, 1:2],
                ).then_inc(vD, 1)
                vector.wait_ge(vD, 3 * B + 4)
                vector.tensor_copy(accb[:], acc[:]).then_inc(vK, 1)
                vector.wait_ge(tS, 2)
                vector.reciprocal(rcp[:], SW[:]).then_inc(vD, 1)
                vector.wait_ge(vD, 3 * B + 5)
                vector.tensor_tensor(
                    res[:], SK[:], rcp[:], op=ALU.mult
                ).then_inc(vF, 1)

            @block.tensor
            def _(tensor):
                tensor.wait_ge(vR, 1)
                tensor.matmul(NM[:], BO[:], ps[:]).then_inc(tN, 1)
                tensor.wait_ge(vK, 1)
                tensor.matmul(SW[:], accb[:, 0:1], MK[:]).then_inc(tS, 1)
                tensor.matmul(SK[:], accb[:, 1:2], MK[:]).then_inc(tS, 1)

    nc.compile()
    return nc


def kernel(cdr3_features=None, all_atom_features=None, **_unused):
    from concourse.bass_utils import run_bass_kernel_spmd

    global last_results
    if "nc" not in _cache:
        _cache["nc"] = _build()
    nc = _cache["nc"]

    x = np.ascontiguousarray(np.asarray(all_atom_features, dtype=np.float32)).reshape(
        P, COLS
    )
    in_map = {"x": x}

    trace = bool(os.environ.get("KERNEL_TRACE"))
    last_results = run_bass_kernel_spmd(
        nc, [in_map] * N_CORES, list(range(N_CORES)), trace=trace
    )
    out = np.asarray(last_results.results[0]["out"], dtype=np.float32)
    return out.reshape(B, 1)
